# revision 16
# baseline (speedup 1.0000x reference)
"""Trainium2 Bass kernel for batched 3x3 VALID conv (NCHW / OIHW).

x: [32, 128, 64, 64] f32, weight: [256, 128, 3, 3] f32 -> out: [32, 256, 62, 62] f32.

Strategy: data-parallel over batch across 8 NeuronCores (4 images each).
Inputs are cast to bf16 on the host (PE rate is identical to fp32r, but
input DMA bytes halve; PSUM accumulation stays fp32 so the error is
~0.4% << the 2e-2 gate). Per core the conv is 9 shift-matmuls
accumulated in PSUM over 62-wide strided windows (no garbage columns):
  out[co, y, x] += W[dy,dx][ci,co].T @ x[ci, y+dy, x+dx]
Output rows are split into 8 groups (7x8 + 1x6 rows); each group's
[co, nr, 62] accumulator fills one PSUM bank. Groups run in halves of
4 with a tap-outer loop (one weight load per tap serving 4 matmuls),
so 4 banks accumulate while the previous 4 drain through the vector
engine to SBUF and out via alternating sync/gpsimd DMA queues.
The first half-block runs group-major so the PE starts as soon as
tap 0 plus ten image rows have landed, consuming weight taps in DMA
arrival order off the (fastest-starting) sync queue.
"""

import numpy as np

_B, _CIN, _H, _W = 32, 128, 64, 64
_COUT = 256
_HO, _WO = 62, 62
_NCORES = 8
_BPC = _B // _NCORES  # images per core
_TAPS = 9

_GROUPS_A = [(0, 8), (8, 8), (16, 8), (24, 8)]
_GROUPS_B = [(32, 8), (40, 8), (48, 8), (56, 6)]

_nc_cache = None


def _build():
    global _nc_cache
    if _nc_cache is not None:
        return _nc_cache

    import concourse.bass as bass
    import concourse.mybir as mybir
    from concourse import bacc
    from concourse.tile import TileContext

    f32 = mybir.dt.float32
    bf16 = mybir.dt.bfloat16

    nc = bacc.Bacc("TRN2", target_bir_lowering=False)
    x_d = nc.dram_tensor("x", [_BPC, _CIN, _H, _W], bf16, kind="ExternalInput")
    w_d = nc.dram_tensor("w", [_CIN, _TAPS, _COUT], bf16, kind="ExternalInput")
    o_d = nc.dram_tensor("o", [_BPC, _COUT, _HO, _WO], f32, kind="ExternalOutput")

    with TileContext(nc) as tc:
        with (
            tc.tile_pool(name="wpool", bufs=1) as wpool,
            tc.tile_pool(name="xpool", bufs=2) as xpool,
            tc.tile_pool(name="spool", bufs=6) as spool,
            tc.tile_pool(name="pspool", bufs=8, space=bass.MemorySpace.PSUM) as pspool,
        ):
            w_sb = wpool.tile([_CIN, _TAPS, _COUT], bf16)
            x_tile_a = xpool.tile([_CIN, _H, _W], bf16, tag="x")
            x_tile_b = xpool.tile([_CIN, _H, _W], bf16, tag="x")
            x_tiles = [x_tile_a, x_tile_b]

            # PE warmup on a zeroed bf16 tile: keeps the PE continuously
            # busy from the end of the prologue so the HAM clock (which
            # also gates DMA throughput) ramps to full speed before the
            # real matmuls and bulk DMA traffic need it. Big 512-row
            # matmuls, sized to end about when the head DMAs land.
            wup = wpool.tile([128, 512], bf16)
            wps = pspool.tile([128, 512], f32, tag="ps")
            dummy = wpool.tile([128, 512], bf16)
            nc.vector.memset(wup[:], 0)
            # Extra element-wise busy-work on the vector engine alongside
            # the PE warmups: more engine activity pushes the HAM power
            # ramp (which also gates DMA throughput) to full speed sooner.
            for _ in range(12):
                nc.vector.tensor_copy(dummy[:], wup[:])
            for _ in range(8):
                nc.tensor.matmul(wps[:], wup[:, 0:128], wup[:], start=True, stop=True)
            # Tapering tail of small warmups: keeps the PE busy (no HAM
            # down-throttle) while the head DMAs finish, with fine enough
            # granularity that the first real matmul slots in quickly.
            for _ in range(16):
                nc.tensor.matmul(
                    wps[:, 0:128], wup[:, 0:128], wup[:, 0:128], start=True, stop=True
                )

            # Head DMAs. DMA throughput depends on per-partition line size,
            # so few big chunks beat many small ones. The sync queue spins
            # up first and carries the weights (two chunks so tap 0 lands
            # early); scalar + gpsimd split img0's rows. Image prefetches
            # ride behind on the scalar queue only, so their WAR waits never
            # block output stores (which rotate sync/gpsimd).
            nc.sync.dma_start(w_sb[:, 0:4, :], w_d[:, 0:4, :])
            nc.gpsimd.dma_start(w_sb[:, 4:9, :], w_d[:, 4:9, :])
            nc.scalar.dma_start(x_tiles[0][:, 0:10, :], x_d[0, :, 0:10, :])
            nc.scalar.dma_start(x_tiles[0][:, 10:22, :], x_d[0, :, 10:22, :])
            nc.scalar.dma_start(x_tiles[0][:, 22:34, :], x_d[0, :, 22:34, :])
            nc.sync.dma_start(x_tiles[0][:, 34:50, :], x_d[0, :, 34:50, :])
            nc.gpsimd.dma_start(x_tiles[0][:, 50:64, :], x_d[0, :, 50:64, :])
            # img1 prefetch (into the b tile) behind img0's scalar chunks.
            nc.scalar.dma_start(x_tiles[1][:], x_d[1])

            store_ctr = [0]

            def mm(ps, x_sb, ct, r0, nr, tap, start, stop):
                dy, dx = divmod(tap, 3)
                nc.tensor.matmul(
                    ps[:, 0:nr, :],
                    w_sb[:, tap, ct * 128 : (ct + 1) * 128],
                    x_sb[:, r0 + dy : r0 + dy + nr, dx : dx + _WO],
                    start=start,
                    stop=stop,
                )

            def drain(ps, img, ct, r0, nr, copy_eng, st_queue):
                st = spool.tile([128, nr, _WO], f32, tag="st")
                o_slice = o_d[img, ct * 128 : (ct + 1) * 128, r0 : r0 + nr, :]
                copy_eng(st[:], ps[:, 0:nr, :])
                st_queue.dma_start(o_slice, st[:])

            for img in range(_BPC):
                x_sb = x_tiles[img % 2]
                for ct in range(_COUT // 128):
                    # Prefetch image img+1 early (img1 already issued above).
                    if ct == 0 and 2 <= img + 1 < _BPC:
                        nc.scalar.dma_start(x_tiles[(img + 1) % 2][:], x_d[img + 1])
                    for half, groups in enumerate((_GROUPS_A, _GROUPS_B)):
                        ps_l = [
                            pspool.tile([128, nr, _WO], f32, tag="ps", name="ps")
                            for (r0, nr) in groups
                        ]
                        head_half = img == 0 and ct == 0 and half == 0
                        tail_half = img == _BPC - 1 and ct == 1 and half == 1
                        if head_half or tail_half:
                            # Head: group-major so the PE starts on tap 0 as
                            # soon as it and the first rows land. Tail:
                            # group-major so the first three groups finish
                            # (and drain) while the last still accumulates.
                            for ps, (r0, nr) in zip(ps_l, groups):
                                for tap in range(_TAPS):
                                    mm(ps, x_sb, ct, r0, nr, tap,
                                       start=(tap == 0), stop=(tap == _TAPS - 1))
                        else:
                            # Steady state: tap-outer, one weight load feeds
                            # four matmuls into four PSUM banks.
                            for tap in range(_TAPS):
                                for ps, (r0, nr) in zip(ps_l, groups):
                                    mm(ps, x_sb, ct, r0, nr, tap,
                                       start=(tap == 0), stop=(tap == _TAPS - 1))
                        late = img == _BPC - 1 and ct == 1
                        for gi, (ps, (r0, nr)) in enumerate(zip(ps_l, groups)):
                            if tail_half and gi >= len(groups) - 2:
                                # Final two groups: halve the copy across
                                # vector+scalar and the store across
                                # sync+scalar so the very last bytes drain
                                # through two engines and two queues.
                                st = spool.tile([128, nr, _WO], f32, tag="st")
                                o_sl = o_d[img, ct * 128 : (ct + 1) * 128, r0 : r0 + nr, :]
                                h = nr // 2
                                nc.vector.tensor_copy(st[:, 0:h, :], ps[:, 0:h, :])
                                nc.scalar.copy(st[:, h:nr, :], ps[:, h:nr, :])
                                nc.sync.dma_start(o_sl[:, 0:h, :], st[:, 0:h, :])
                                nc.scalar.dma_start(o_sl[:, h:nr, :], st[:, h:nr, :])
                            elif tail_half:
                                # Earlier tail groups finish 1.9us apart
                                # (group-major): drain them under the
                                # remaining matmuls, off the slow gpsimd
                                # queue.
                                copy_eng = (
                                    nc.vector.tensor_copy
                                    if gi % 2 == 0
                                    else nc.scalar.copy
                                )
                                q = nc.sync if gi % 2 == 0 else nc.scalar
                                drain(ps, img, ct, r0, nr, copy_eng, q)
                            elif late:
                                drain(ps, img, ct, r0, nr, nc.vector.tensor_copy, nc.sync)
                            else:
                                q = nc.sync if store_ctr[0] % 2 == 0 else nc.gpsimd
                                store_ctr[0] += 1
                                drain(ps, img, ct, r0, nr, nc.vector.tensor_copy, q)

    nc.compile()
    _nc_cache = nc
    return nc


def _prep_in_maps(x, weight):
    from concourse import mybir

    np_bf16 = mybir.dt.np(mybir.dt.bfloat16)
    x = np.asarray(x, dtype=np.float32)
    w = np.asarray(weight, dtype=np.float32)
    assert x.shape == (_B, _CIN, _H, _W), x.shape
    assert w.shape == (_COUT, _CIN, 3, 3), w.shape
    # w[ci, dy*3+dx, co] = weight[co, ci, dy, dx]
    wt = np.ascontiguousarray(
        w.transpose(1, 2, 3, 0).reshape(_CIN, _TAPS, _COUT)
    ).astype(np_bf16)
    xs = x.reshape(_NCORES, _BPC, _CIN, _H, _W).astype(np_bf16)
    return [{"x": np.ascontiguousarray(xs[i]), "w": wt} for i in range(_NCORES)]


def _run(x, weight, **kwargs):
    from concourse.bass_utils import run_bass_kernel_spmd

    nc = _build()
    res = run_bass_kernel_spmd(
        nc, _prep_in_maps(x, weight), core_ids=list(range(_NCORES)), **kwargs
    )
    out = np.concatenate([r["o"] for r in res.results], axis=0)
    return out.astype(np.float32, copy=False), res


def kernel(x, weight):
    out, _ = _run(x, weight)
    return out


# revision 19
# speedup vs baseline: 1.2079x; 1.2079x over previous
"""Trainium2 Bass kernel for batched 3x3 VALID conv (NCHW / OIHW).

x: [32, 128, 64, 64] f32, weight: [256, 128, 3, 3] f32 -> out: [32, 256, 62, 62] f32.

Strategy: data-parallel over batch across 8 NeuronCores (4 images each).
Inputs are cast to bf16 on the host (PE rate is identical to fp32r, but
input DMA bytes halve; PSUM accumulation stays fp32 so the error is
~0.4% << the 2e-2 gate). Per core the conv is 9 shift-matmuls
accumulated in PSUM over 62-wide strided windows (no garbage columns):
  out[co, y, x] += W[dy,dx][ci,co].T @ x[ci, y+dy, x+dx]
Output rows are split into 8 groups (7x8 + 1x6 rows); each group's
[co, nr, 62] accumulator fills one PSUM bank. Groups run in halves of
4 with a tap-outer loop (one weight load per tap serving 4 matmuls),
so 4 banks accumulate while the previous 4 drain through the vector
engine to SBUF and out via alternating sync/gpsimd DMA queues.
The first half-block runs group-major so the PE starts as soon as
tap 0 plus ten image rows have landed, consuming weight taps in DMA
arrival order off the (fastest-starting) sync queue.
"""

import numpy as np

_B, _CIN, _H, _W = 32, 128, 64, 64
_COUT = 256
_HO, _WO = 62, 62
_NCORES = 8
_BPC = _B // _NCORES  # images per core
_TAPS = 9

_GROUPS_A = [(0, 8), (8, 8), (16, 8), (24, 8)]
_GROUPS_B = [(32, 8), (40, 8), (48, 8), (56, 6)]
# Last block only: split the 6-row group so the final PSUM bank is tiny
# and the end-of-kernel drain (copy + descriptor + store + completion
# semaphore) covers 2 rows instead of 6.
_GROUPS_B_TAIL = [(32, 8), (40, 8), (48, 8), (56, 4), (60, 2)]

_nc_cache = None


def _build():
    global _nc_cache
    if _nc_cache is not None:
        return _nc_cache

    import concourse.bass as bass
    import concourse.mybir as mybir
    from concourse import bacc
    from concourse.tile import TileContext

    f32 = mybir.dt.float32
    bf16 = mybir.dt.bfloat16

    nc = bacc.Bacc("TRN2", target_bir_lowering=False)
    x_d = nc.dram_tensor("x", [_BPC, _CIN, _H, _W], bf16, kind="ExternalInput")
    w_d = nc.dram_tensor("w", [_CIN, _TAPS, _COUT], bf16, kind="ExternalInput")
    o_d = nc.dram_tensor("o", [_BPC, _COUT, _HO, _WO], f32, kind="ExternalOutput")

    with TileContext(nc) as tc:
        with (
            tc.tile_pool(name="wpool", bufs=1) as wpool,
            tc.tile_pool(name="xpool", bufs=2) as xpool,
            tc.tile_pool(name="spool", bufs=6) as spool,
            tc.tile_pool(name="pspool", bufs=8, space=bass.MemorySpace.PSUM) as pspool,
        ):
            w_sb = wpool.tile([_CIN, _TAPS, _COUT], bf16)
            x_tile_a = xpool.tile([_CIN, _H, _W], bf16, tag="x")
            x_tile_b = xpool.tile([_CIN, _H, _W], bf16, tag="x")
            x_tiles = [x_tile_a, x_tile_b]

            # PE warmup on a zeroed bf16 tile: keeps the PE continuously
            # busy from the end of the prologue so the HAM clock (which
            # also gates DMA throughput) ramps to full speed before the
            # real matmuls and bulk DMA traffic need it. Big 512-row
            # matmuls, sized to end about when the head DMAs land.
            wup = wpool.tile([128, 512], bf16)
            wps = pspool.tile([128, 512], f32, tag="ps")
            dummy = wpool.tile([128, 512], bf16)
            nc.vector.memset(wup[:], 0)
            # Extra element-wise busy-work on the vector engine alongside
            # the PE warmups: more engine activity pushes the HAM power
            # ramp (which also gates DMA throughput) to full speed sooner.
            for _ in range(12):
                nc.vector.tensor_copy(dummy[:], wup[:])
            for _ in range(8):
                nc.tensor.matmul(wps[:], wup[:, 0:128], wup[:], start=True, stop=True)
            # Tapering tail of small warmups: keeps the PE busy (no HAM
            # down-throttle) while the head DMAs finish, with fine enough
            # granularity that the first real matmul slots in quickly.
            for _ in range(16):
                nc.tensor.matmul(
                    wps[:, 0:128], wup[:, 0:128], wup[:, 0:128], start=True, stop=True
                )

            # Head DMAs. DMA throughput depends on per-partition line size,
            # so few big chunks beat many small ones. The sync queue spins
            # up first and carries the weights (two chunks so tap 0 lands
            # early); scalar + gpsimd split img0's rows. Image prefetches
            # ride behind on the scalar queue only, so their WAR waits never
            # block output stores (which rotate sync/gpsimd).
            nc.sync.dma_start(w_sb[:, 0:4, :], w_d[:, 0:4, :])
            nc.gpsimd.dma_start(w_sb[:, 4:9, :], w_d[:, 4:9, :])
            nc.scalar.dma_start(x_tiles[0][:, 0:10, :], x_d[0, :, 0:10, :])
            nc.scalar.dma_start(x_tiles[0][:, 10:22, :], x_d[0, :, 10:22, :])
            nc.scalar.dma_start(x_tiles[0][:, 22:34, :], x_d[0, :, 22:34, :])
            nc.sync.dma_start(x_tiles[0][:, 34:50, :], x_d[0, :, 34:50, :])
            nc.gpsimd.dma_start(x_tiles[0][:, 50:64, :], x_d[0, :, 50:64, :])
            # img1 prefetch (into the b tile) behind img0's scalar chunks.
            nc.scalar.dma_start(x_tiles[1][:], x_d[1])

            store_ctr = [0]

            def mm(ps, x_sb, ct, r0, nr, tap, start, stop):
                dy, dx = divmod(tap, 3)
                nc.tensor.matmul(
                    ps[:, 0:nr, :],
                    w_sb[:, tap, ct * 128 : (ct + 1) * 128],
                    x_sb[:, r0 + dy : r0 + dy + nr, dx : dx + _WO],
                    start=start,
                    stop=stop,
                )

            def drain(ps, img, ct, r0, nr, copy_eng, st_queue):
                st = spool.tile([128, nr, _WO], f32, tag="st")
                o_slice = o_d[img, ct * 128 : (ct + 1) * 128, r0 : r0 + nr, :]
                copy_eng(st[:], ps[:, 0:nr, :])
                st_queue.dma_start(o_slice, st[:])

            for img in range(_BPC):
                x_sb = x_tiles[img % 2]
                for ct in range(_COUT // 128):
                    # Prefetch image img+1 early (img1 already issued above).
                    if ct == 0 and 2 <= img + 1 < _BPC:
                        nc.scalar.dma_start(x_tiles[(img + 1) % 2][:], x_d[img + 1])
                    for half, groups in enumerate((_GROUPS_A, _GROUPS_B)):
                        head_half = img == 0 and ct == 0 and half == 0
                        tail_half = img == _BPC - 1 and ct == 1 and half == 1
                        if tail_half:
                            groups = _GROUPS_B_TAIL
                        ps_l = [
                            pspool.tile([128, nr, _WO], f32, tag="ps", name="ps")
                            for (r0, nr) in groups
                        ]
                        if head_half or tail_half:
                            # Head: group-major so the PE starts on tap 0 as
                            # soon as it and the first rows land. Tail:
                            # group-major so the first three groups finish
                            # (and drain) while the last still accumulates.
                            for ps, (r0, nr) in zip(ps_l, groups):
                                for tap in range(_TAPS):
                                    mm(ps, x_sb, ct, r0, nr, tap,
                                       start=(tap == 0), stop=(tap == _TAPS - 1))
                        else:
                            # Steady state: tap-outer, one weight load feeds
                            # four matmuls into four PSUM banks.
                            for tap in range(_TAPS):
                                for ps, (r0, nr) in zip(ps_l, groups):
                                    mm(ps, x_sb, ct, r0, nr, tap,
                                       start=(tap == 0), stop=(tap == _TAPS - 1))
                        late = img == _BPC - 1 and ct == 1
                        for gi, (ps, (r0, nr)) in enumerate(zip(ps_l, groups)):
                            if tail_half and gi == len(groups) - 2:
                                # 4-row group: drains on scalar engine+queue
                                # while the 2-row group still accumulates.
                                drain(ps, img, ct, r0, nr, nc.scalar.copy, nc.scalar)
                            elif tail_half and gi == len(groups) - 1:
                                # Final 2-row group: vector copy + sync
                                # store, the shortest possible last drain.
                                drain(ps, img, ct, r0, nr, nc.vector.tensor_copy, nc.sync)
                            elif tail_half:
                                # Earlier tail groups finish 1.9us apart
                                # (group-major): drain them under the
                                # remaining matmuls, off the slow gpsimd
                                # queue.
                                copy_eng = (
                                    nc.vector.tensor_copy
                                    if gi % 2 == 0
                                    else nc.scalar.copy
                                )
                                q = nc.sync if gi % 2 == 0 else nc.scalar
                                drain(ps, img, ct, r0, nr, copy_eng, q)
                            elif late:
                                drain(ps, img, ct, r0, nr, nc.vector.tensor_copy, nc.sync)
                            else:
                                q = nc.sync if store_ctr[0] % 2 == 0 else nc.gpsimd
                                store_ctr[0] += 1
                                drain(ps, img, ct, r0, nr, nc.vector.tensor_copy, q)

    nc.compile()
    _nc_cache = nc
    return nc


def _prep_in_maps(x, weight):
    from concourse import mybir

    np_bf16 = mybir.dt.np(mybir.dt.bfloat16)
    x = np.asarray(x, dtype=np.float32)
    w = np.asarray(weight, dtype=np.float32)
    assert x.shape == (_B, _CIN, _H, _W), x.shape
    assert w.shape == (_COUT, _CIN, 3, 3), w.shape
    # w[ci, dy*3+dx, co] = weight[co, ci, dy, dx]
    wt = np.ascontiguousarray(
        w.transpose(1, 2, 3, 0).reshape(_CIN, _TAPS, _COUT)
    ).astype(np_bf16)
    xs = x.reshape(_NCORES, _BPC, _CIN, _H, _W).astype(np_bf16)
    return [{"x": np.ascontiguousarray(xs[i]), "w": wt} for i in range(_NCORES)]


def _run(x, weight, **kwargs):
    from concourse.bass_utils import run_bass_kernel_spmd

    nc = _build()
    res = run_bass_kernel_spmd(
        nc, _prep_in_maps(x, weight), core_ids=list(range(_NCORES)), **kwargs
    )
    out = np.concatenate([r["o"] for r in res.results], axis=0)
    return out.astype(np.float32, copy=False), res


def kernel(x, weight):
    out, _ = _run(x, weight)
    return out
